# revision 6
# baseline (speedup 1.0000x reference)
"""Trainium2 Bass kernel for nn_ContextLabel (GNN label propagation).

Computation: 10 iterations of Y = masked(adj @ Y) on [10000,16], then
straight-through gumbel one-hot, dist = (adj!=0) @ Yh row-normalized,
output mean((dist - pseudo_labels)^2)  (scalar).

Strategy (8 NeuronCores, row-parallel):
 - core c owns rows [1250c, 1250c+1250)
 - adj^T shard (fp8 e4m3, [10000 x 1250]) stays RESIDENT in SBUF; all 10
   propagation passes stream it from SBUF through the tensor engine with
   Y (fp16) as the stationary operand: out^T[16,1250] = Y^T @ adjT.
 - per-iteration AllGather of the fp16 Y slice across the 8 cores.
 - final pass streams the 0/1 mask (fp8) from HBM, Yh (fp8 one-hot)
   stationary; row-normalize + squared-error partials on device.
fp8 adj values give ~1 argmax flip out of 10000 rows (verified on host:
final relerr ~3e-5); Y in fp16 is bit-exact vs fp32 for the argmax.
"""

import hashlib
import os
import shutil
import sys
from pathlib import Path

import numpy as np
import ml_dtypes

sys.path.insert(0, "/opt/trn_rl_repo")

import concourse.bass as bass  # noqa: E402
import concourse.mybir as mybir  # noqa: E402
import concourse.tile as tile  # noqa: E402
from concourse import bacc  # noqa: E402
import concourse.bass2jax as bass2jax  # noqa: E402
from concourse.bass_utils import run_bass_kernel_spmd  # noqa: E402
from concourse.masks import make_identity  # noqa: E402

F8 = ml_dtypes.float8_e4m3
NCORES = 8
N = 10000
C = 16
R = N // NCORES           # 1250 rows per core
NBLK = R // 128           # 10 blocks of local rows (1250 = 9*128 + 82 -> no!)
# careful: 1250 = 9*128 + 82; use 10 blocks of 125? No - we transpose in
# [16,128] slices; 1250 splits as 9 full 128-slices + one 82-slice.
FULLB = R // 128          # 9 full 128-col blocks
TAILB = R - FULLB * 128   # 82
KCH = 78                  # full 128-row contraction chunks
KTAIL = N - KCH * 128     # 16
NCHT = KCH + 1            # 79 chunk slots in tiled [128, 79*16] layout
SL = [(0, 512), (512, 512), (1024, 226)]  # psum bank slices of 1250
MGRP = 8                  # mask-stream chunks per DMA group

_NEFF_CACHE = Path.home() / ".cache" / "bass_neff"


def _install_neff_cache():
    orig = bass2jax.compile_bir_kernel
    if getattr(bass2jax.compile_bir_kernel, "_cached", False):
        return

    def cached(bir_json, tmpdir, neff_name="file.neff"):
        h = hashlib.sha256(bir_json).hexdigest()
        p = _NEFF_CACHE / f"{h}.neff"
        dst = os.path.join(tmpdir, neff_name)
        if p.exists():
            shutil.copy(p, dst)
            return dst
        out = orig(bir_json, tmpdir, neff_name)
        try:
            _NEFF_CACHE.mkdir(parents=True, exist_ok=True)
            shutil.copy(out, p)
        except OSError:
            pass
        return out

    cached._cached = True
    bass2jax.compile_bir_kernel = cached


def build_program():
    nc = bacc.Bacc(
        "TRN2", target_bir_lowering=False, debug=False,
        enable_asserts=False, num_devices=NCORES,
    )
    f8, f16, f32 = mybir.dt.float8e4, mybir.dt.float16, mybir.dt.float32

    adjT_d = nc.dram_tensor("adjT8", [N, R], f8, kind="ExternalInput")
    maskT_d = nc.dram_tensor("maskT8", [N, R], f8, kind="ExternalInput")
    gumt_d = nc.dram_tensor("gumt", [128, NCHT * C], f32, kind="ExternalInput")
    labmt_d = nc.dram_tensor("labmt", [128, NCHT * C], f16, kind="ExternalInput")
    m16t_d = nc.dram_tensor("m16t", [128, NCHT * C], mybir.dt.uint8, kind="ExternalInput")
    mT16_d = nc.dram_tensor("mT16", [C, R], mybir.dt.uint8, kind="ExternalInput")
    labT_d = nc.dram_tensor("labT", [C, R], f16, kind="ExternalInput")
    pst_d = nc.dram_tensor("pst", [128, FULLB + 1, C], f32, kind="ExternalInput")
    out_d = nc.dram_tensor("out_sq", [128, FULLB + 1], f32, kind="ExternalOutput")

    with tile.TileContext(nc) as tc:
        with (
            tc.tile_pool(name="sb", bufs=1) as sb,
            tc.tile_pool(name="mtp", bufs=2) as mtp,
            tc.tile_pool(name="ps", bufs=2, space="PSUM") as ps,
            tc.tile_pool(name="dram", bufs=2, space="DRAM") as dram,
        ):
            # ---- resident tiles -------------------------------------
            at_g = []
            for g in range(10):
                kc = 8 if g < 9 else 6
                t = sb.tile([128, kc * R], f8, name=f"at{g}", tag=f"at{g}")
                at_g.append(t)
            at_last = sb.tile([KTAIL, R], f8)
            ycur = sb.tile([128, NCHT * C], f16)
            gumt = sb.tile([128, NCHT * C], f32)
            labmt = sb.tile([128, NCHT * C], f16)
            m16t = sb.tile([128, NCHT * C], mybir.dt.uint8)
            mT16 = sb.tile([C, R], mybir.dt.uint8)
            labT = sb.tile([C, R], f16)
            pst = sb.tile([128, FULLB + 1, C], f32)
            ident = sb.tile([C, C], f16)
            yT = sb.tile([C, R], f16)
            yloc = sb.tile([128, FULLB + 1, C], f16)
            logits = sb.tile([128, NCHT, C], f32)
            rmax = sb.tile([128, NCHT], f32)
            yh16 = sb.tile([128, NCHT * C], f16)
            yh8 = sb.tile([128, NCHT * C], f8)

            make_identity(nc, ident[:])

            # ---- initial loads --------------------------------------
            for g in range(10):
                kc = 8 if g < 9 else 6
                src = adjT_d[g * 8 * 128:(g * 8 + kc) * 128, :]
                nc.sync.dma_start(
                    out=at_g[g][:].rearrange("p (k i) -> p k i", k=kc),
                    in_=src.rearrange("(k p) i -> p k i", p=128),
                )
            nc.sync.dma_start(out=at_last[:], in_=adjT_d[KCH * 128:N, :])
            nc.sync.dma_start(out=ycur[:], in_=labmt_d[:])  # Y0 = labels*m
            nc.sync.dma_start(out=gumt[:], in_=gumt_d[:])
            nc.sync.dma_start(out=labmt[:], in_=labmt_d[:])
            nc.sync.dma_start(out=m16t[:], in_=m16t_d[:])
            nc.sync.dma_start(out=mT16[:], in_=mT16_d[:])
            nc.sync.dma_start(out=labT[:], in_=labT_d[:])
            nc.sync.dma_start(out=pst[:], in_=pst_d[:])

            def mm_pass(acc, lhs_tile, rhs_chunk):
                """acc[16,1250] += lhs.T @ rhsT  over all 79 chunks."""
                for k in range(NCHT):
                    if k < KCH:
                        lhsT = lhs_tile[:, k * C:(k + 1) * C]
                        rhs_t, rhs_off = rhs_chunk(k)
                    else:
                        lhsT = lhs_tile[0:KTAIL, k * C:(k + 1) * C]
                        rhs_t, rhs_off = rhs_chunk(k)
                    for (s0, sw) in SL:
                        nc.tensor.matmul(
                            acc[:, s0:s0 + sw],
                            lhsT,
                            rhs_t[0:lhsT.partition_size(),
                                  rhs_off + s0:rhs_off + s0 + sw],
                            start=(k == 0), stop=(k == NCHT - 1),
                        )

            def at_chunk(k):
                if k < KCH:
                    g, j = k // 8, k % 8
                    return at_g[g], j * R
                return at_last, 0

            # ---- 10 propagation iterations --------------------------
            for t in range(10):
                acc = ps.tile([C, R], f32, name=f"acc{t}", tag="acc")
                mm_pass(acc, ycur, at_chunk)
                # masked overwrite in transposed layout, cast to fp16
                nc.vector.tensor_copy(yT[:], acc[:])
                nc.vector.copy_predicated(yT[:], mT16[:], labT[:])
                # transpose [16,1250] -> [1250,16] via PE, in 128-col blocks
                trp = ps.tile([128, (FULLB + 1) * C], f16, name=f"trp{t}", tag="trp")
                for b in range(FULLB):
                    nc.tensor.transpose(
                        trp[:, b * C:(b + 1) * C],
                        yT[:, b * 128:(b + 1) * 128], ident[:],
                    )
                nc.tensor.transpose(
                    trp[0:TAILB, FULLB * C:(FULLB + 1) * C],
                    yT[:, FULLB * 128:R], ident[:],
                )
                nc.vector.tensor_copy(yloc[:].rearrange("p b c -> p (b c)"), trp[:])
                # exchange local rows -> full Y
                cc_in = dram.tile([R, C], f16, name=f"ccin{t}", tag="ccin")
                cc_out = dram.tile([N, C], f16, name=f"ccout{t}", tag="ccout",
                                   addr_space="Shared")
                nc.sync.dma_start(
                    out=cc_in[0:FULLB * 128, :].rearrange("(b p) c -> p b c", p=128),
                    in_=yloc[0:128, 0:FULLB, :],
                )
                nc.sync.dma_start(
                    out=cc_in[FULLB * 128:R, :],
                    in_=yloc[0:TAILB, FULLB, :],
                )
                nc.gpsimd.collective_compute(
                    "AllGather", mybir.AluOpType.bypass,
                    replica_groups=[list(range(NCORES))],
                    ins=[cc_in[:]], outs=[cc_out[:]],
                )
                nc.sync.dma_start(
                    out=ycur[:, 0:KCH * C].rearrange("p (k c) -> p k c", c=C),
                    in_=cc_out[0:KCH * 128, :].rearrange("(k p) c -> p k c", p=128),
                )
                nc.sync.dma_start(
                    out=ycur[0:KTAIL, KCH * C:NCHT * C],
                    in_=cc_out[KCH * 128:N, :],
                )

            # ---- straight-through gumbel one-hot --------------------
            nc.vector.tensor_tensor(
                logits[:].rearrange("p k c -> p (k c)"), ycur[:], gumt[:],
                mybir.AluOpType.add,
            )
            nc.vector.tensor_reduce(
                rmax[:], logits[:], axis=mybir.AxisListType.X,
                op=mybir.AluOpType.max,
            )
            nc.vector.tensor_tensor(
                yh16[:].rearrange("p (k c) -> p k c", c=C),
                logits[:],
                rmax[:].unsqueeze(2).broadcast_to([128, NCHT, C]),
                mybir.AluOpType.is_equal,
            )
            nc.vector.copy_predicated(yh16[:], m16t[:], labmt[:])
            nc.vector.tensor_copy(yh8[:], yh16[:])

            # ---- final pass: dist^T = Yh^T @ maskT ------------------
            mt_tiles = {}
            for g in range(10):
                kc = 8 if g < 9 else 6
                mt = mtp.tile([128, kc * R], f8, name=f"mt{g}", tag="mt")
                src = maskT_d[g * 8 * 128:(g * 8 + kc) * 128, :]
                nc.sync.dma_start(
                    out=mt[:].rearrange("p (k i) -> p k i", k=kc),
                    in_=src.rearrange("(k p) i -> p k i", p=128),
                )
                mt_tiles[g] = mt
            mt_last = sb.tile([KTAIL, R], f8)
            nc.sync.dma_start(out=mt_last[:], in_=maskT_d[KCH * 128:N, :])

            def mt_chunk(k):
                if k < KCH:
                    g, j = k // 8, k % 8
                    return mt_tiles[g], j * R
                return mt_last, 0

            dacc = ps.tile([C, R], f32, tag="acc")
            mm_pass(dacc, yh8, mt_chunk)

            # ---- normalize + squared error --------------------------
            dT = sb.tile([C, R], f32)
            nc.vector.tensor_copy(dT[:], dacc[:])
            ident32 = sb.tile([C, C], f32)
            make_identity(nc, ident32[:])
            trd = ps.tile([128, (FULLB + 1) * C], f32, tag="trp")
            # tail block: partitions >= TAILB are never written by the
            # transpose; zero them so normalization stays finite
            nc.vector.memset(trd[:, FULLB * C:(FULLB + 1) * C], 0.0)
            for b in range(FULLB):
                nc.tensor.transpose(
                    trd[:, b * C:(b + 1) * C],
                    dT[:, b * 128:(b + 1) * 128], ident32[:],
                )
            nc.tensor.transpose(
                trd[0:TAILB, FULLB * C:(FULLB + 1) * C],
                dT[:, FULLB * 128:R], ident32[:],
            )
            dist = sb.tile([128, FULLB + 1, C], f32)
            nc.vector.tensor_copy(dist[:].rearrange("p b c -> p (b c)"), trd[:])
            rsum = sb.tile([128, FULLB + 1], f32)
            nc.vector.tensor_reduce(
                rsum[:], dist[:], axis=mybir.AxisListType.X,
                op=mybir.AluOpType.add,
            )
            rinv = sb.tile([128, FULLB + 1], f32)
            # valid rows always have rsum >= 1 (self-loop); clamp the zeroed
            # tail-garbage rows so 1/rsum stays finite (their dist is 0)
            nc.vector.tensor_scalar_max(rsum[:], rsum[:], 0.5)
            nc.vector.reciprocal(rinv[:], rsum[:])
            dd = sb.tile([128, FULLB + 1, C], f32)
            nc.vector.tensor_tensor(
                dd[:], dist[:],
                rinv[:].unsqueeze(2).broadcast_to([128, FULLB + 1, C]),
                mybir.AluOpType.mult,
            )
            nc.vector.tensor_tensor(dd[:], dd[:], pst[:], mybir.AluOpType.subtract)
            nc.vector.tensor_tensor(dd[:], dd[:], dd[:], mybir.AluOpType.mult)
            osq = sb.tile([128, FULLB + 1], f32)
            nc.vector.tensor_reduce(
                osq[:], dd[:], axis=mybir.AxisListType.X, op=mybir.AluOpType.add,
            )
            nc.sync.dma_start(out=out_d[:], in_=osq[:])

    nc.compile()
    return nc


_nc = None


def _get_program():
    global _nc
    if _nc is None:
        _install_neff_cache()
        _nc = build_program()
    return _nc


def prep_inputs(adj, labels_onehot, pseudo_labels, gumbel, train_mask):
    adj = np.asarray(adj, np.float32)
    labels = np.asarray(labels_onehot, np.float32)
    pseudo = np.asarray(pseudo_labels, np.float32)
    gumbel = np.asarray(gumbel, np.float32)
    m = np.asarray(train_mask).astype(bool)

    def tile_full(x, dtype):
        """[N,cols] -> [128, 79*cols] chunk-tiled, zero-padded."""
        cols = x.shape[1]
        p = np.zeros((NCHT * 128, cols), x.dtype)
        p[:N] = x
        return np.ascontiguousarray(
            p.reshape(NCHT, 128, cols).transpose(1, 0, 2).reshape(128, NCHT * cols)
        ).astype(dtype)

    labm = labels * m[:, None]
    gumt = tile_full(gumbel, np.float32)
    labmt = tile_full(labm, np.float16)
    m16 = np.repeat(m[:, None].astype(np.uint8), C, axis=1)
    m16t = tile_full(m16, np.uint8)

    in_maps = []
    for c in range(NCORES):
        rows = slice(c * R, (c + 1) * R)
        blk = np.ascontiguousarray(adj[rows, :].T)          # [N, R]
        adjT8 = blk.astype(F8)
        maskT8 = (blk != 0).astype(F8)
        mT16 = np.ascontiguousarray(
            np.broadcast_to(m[rows].astype(np.uint8), (C, R)))
        labT = np.ascontiguousarray(labm[rows].T.astype(np.float16))
        ps_loc = np.zeros(((FULLB + 1) * 128, C), np.float32)
        ps_loc[:R] = pseudo[rows]
        pst = np.ascontiguousarray(
            ps_loc.reshape(FULLB + 1, 128, C).transpose(1, 0, 2))
        in_maps.append({
            "adjT8": adjT8, "maskT8": maskT8, "gumt": gumt,
            "labmt": labmt, "m16t": m16t, "mT16": mT16, "labT": labT,
            "pst": pst,
        })
    return in_maps


def run_on_device(in_maps, trace=False, **kw):
    nc = _get_program()
    return run_bass_kernel_spmd(nc, in_maps, list(range(NCORES)), trace=trace, **kw)


def kernel(adj, labels_onehot, pseudo_labels, gumbel, train_mask,
           iter_step=10, k_hop=1, **_unused):
    assert int(iter_step) == 10 and int(k_hop) == 1, "kernel hardcodes 10/1"
    in_maps = prep_inputs(adj, labels_onehot, pseudo_labels, gumbel, train_mask)
    res = run_on_device(in_maps)
    total = 0.0
    for c in range(NCORES):
        sq = np.asarray(res.results[c]["out_sq"], np.float64)
        total += sq.sum()
    return np.float32(total / (N * C))
